# revision 1
# baseline (speedup 1.0000x reference)
"""Trainium2 Bass kernel for nn_MultiHeadDensityRatioEstimator.

Math restructure vs the jax reference:
  logits l_h(i,j) = -log1p(sq_h(i,j))  with sq = ||zy_i||^2+||zx_j||^2-2<zy_i,zx_j>
  exp(l_h) = 1/(1+sq_h) =: w_h   -> every logsumexp becomes a plain sum of w
  sum_h l_h = ln(prod_h w_h)     -> one log per pair instead of 8

Per core (8 cores, zy rows sharded 512/core), the pair matrix is computed
TRANSPOSED — tiles are [128 zx-rows j, 512 zy-rows i] — so the per-(i,h)
row sums over j become PE matmul accumulations into one PSUM tile:
  v_h = 1+sq_h from one K=18 augmented matmul per (head, j-block)  [PSUM]
  w_h = reciprocal_approx_fast(v_h) written bf16                   [DVE]
  rowsums: masked-ones matmuls accumulating [8, 512] in PSUM       [PE]
  savg = ln(prod_h w_h) stored [4096, 512]                [DVE+GPSIMD+ACT Ln]
  tiny AllReduce of the 8 per-head global sums -> baseline blavg
  sigmoid / count / sum sweeps over stored savg
  8 partial stats out per core; host combines to the 9 scalars.
"""

import math
import sys

import numpy as np

for _p in ("/opt/trn_rl_repo",):
    if _p not in sys.path:
        sys.path.insert(0, _p)

N = 4096
D = 128
H = 8
DH = 16
NCORES = 8
RPC = N // NCORES  # rows per core = 512
NIB = RPC // 128  # 4 chunks of this core's rows
NJB = N // 128  # 32 j-blocks of 128
LOG_NN1 = float(np.log(float(N) * (N - 1)))
NSTAT = 8


def build_bass():
    import ml_dtypes
    import concourse.bacc as bacc
    import concourse.tile as tile
    from concourse import masks, mybir
    from concourse.dve_ops import RECIP_APPROX_FAST_CONSTS, RECIPROCAL_APPROX_FAST

    f32 = mybir.dt.float32
    f32r = mybir.dt.float32r
    bf16 = mybir.dt.bfloat16
    AF = mybir.ActivationFunctionType
    ALU = mybir.AluOpType
    AX = mybir.AxisListType
    RC = RECIP_APPROX_FAST_CONSTS

    nc = bacc.Bacc("TRN2", num_devices=NCORES, debug=False)

    zx = nc.dram_tensor("z_x", [N, D], f32, kind="ExternalInput")
    # z_yd[:, 0:128] = this core's zy rows; [:, 128:256] = matching zx rows
    zyd = nc.dram_tensor("z_yd", [RPC, 2 * D], f32, kind="ExternalInput")
    out = nc.dram_tensor("out", [1, NSTAT], f32, kind="ExternalOutput")

    from contextlib import ExitStack

    with tile.TileContext(nc) as tc, ExitStack() as stk:
        # ---------- persistent pools ----------
        big = stk.enter_context(tc.tile_pool(name="big", bufs=1))
        small = stk.enter_context(tc.tile_pool(name="small", bufs=1))

        # packed matmul operands: head h -> tensor HT[h], slot HS[h] (32-part
        # stride; matmul operand base partition must be 0/32/64)
        HT = [0, 0, 0, 1, 1, 1, 2, 2]
        HS = [0, 1, 2, 0, 1, 2, 0, 1]
        RHEADS = [[0, 1, 2], [3, 4, 5], [6, 7]]
        # lhsT side (zx): rows [32s,32s+16) = -2*zx_h^T ; 32s+16 = xn_h+0.5 ;
        # 32s+17 = 1.  rhs side (zy): [32s,32s+16) = zy_h^T ; 32s+16 = 1 ;
        # 32s+17 = yn_h+0.5
        XTA = [big.tile([96, N], f32, tag=f"xta{t}", name=f"XTA{t}") for t in range(3)]
        YTA = [big.tile([96, RPC], f32, tag=f"yta{t}", name=f"YTA{t}") for t in range(3)]
        # stored savg = ln(prod_h w_h); j-block jb lives at
        # Qst[jb % 4][:, (jb//4)*512 : ...]
        Qst = [big.tile([128, 8 * 512], f32, tag=f"qst{t}", name=f"Qst{t}") for t in range(4)]

        ident = small.tile([128, 128], f32)
        vdall = small.tile([128, NIB * H], f32)
        wdall = small.tile([128, NIB * H], f32)
        pd1 = small.tile([128, 16], f32)
        pd2 = small.tile([128, 8], f32)
        pdw = small.tile([128, 4], f32)
        Ldw = small.tile([128, 4], f32)
        stats = small.tile([128, NSTAT], f32)
        slq = small.tile([128, 4], f32)
        ssig = small.tile([128, 4], f32)
        scnt = small.tile([128, 4], f32)
        ones128 = small.tile([128, 1], f32)
        ones1 = small.tile([1, 128], f32)
        ones8 = small.tile([8, 1], f32)
        half8 = small.tile([8, 1], f32)
        Eall = small.tile([128, 8 * H], bf16)
        rsS = small.tile([8, RPC], f32)
        wdT = small.tile([8, RPC], f32)
        Sp = small.tile([8, 1], f32)
        Sg = small.tile([8, 1], f32)
        lnrs_red = small.tile([8, 1], f32)
        blavg_t = small.tile([1, 1], f32)
        repS = small.tile([1, 1], f32)
        nbl = small.tile([128, 1], f32)
        t8b = small.tile([128, 1], f32)
        outrow = small.tile([1, NSTAT], f32)

        nc.vector.memset(ones128[:], 1.0)
        nc.vector.memset(ones1[:], 1.0)
        nc.vector.memset(ones8[:], 1.0)
        nc.vector.memset(half8[:], 0.5)
        nc.vector.memset(stats[:], 0.0)
        masks.make_identity(nc, ident[:])

        # E matrix for rowsum matmuls: Eall[:, h*8+a] = (a == h), bf16
        em = np.zeros((128, 8 * H), np.float32)
        for h in range(H):
            em[:, h * 8 + h] = 1.0
        Ed = nc.inline_tensor(em.astype(ml_dtypes.bfloat16), name="eall_const")

        # ---------- preprocessing: transposes + packed operand assembly ----------
        with (
            tc.tile_pool(name="pp_sbuf", bufs=4) as pp,
            tc.tile_pool(name="pp_keep", bufs=1) as ppk,
            tc.tile_pool(name="pp_psum", bufs=4, space="PSUM") as ppp,
        ):
            X2T = ppk.tile([128, N], f32)  # -2 * zx^T
            YTfull = ppk.tile([128, RPC], f32)  # zy^T
            xna = ppk.tile([8, N], f32)  # xn_h[j] + 0.5
            yna = ppk.tile([8, RPC], f32)  # yn_h[i] + 0.5
            Hmask = ppk.tile([128, 8], f32)

            hm = np.zeros((128, 8), np.float32)
            for h in range(H):
                hm[h * DH : (h + 1) * DH, h] = 1.0
            hmd = nc.inline_tensor(hm, name="hmask_const")
            onesd = nc.inline_tensor(np.ones((1, N), np.float32), name="ones_const")

            # stage full inputs with ONE DMA each (gpsimd = single SWDGE sem)
            SX = ppk.tile([128, N], f32)
            SYD = ppk.tile([128, NIB * 2 * D], f32)
            nc.gpsimd.dma_start(
                out=SX.rearrange("p (t d) -> p t d", d=D),
                in_=zx.rearrange("(t p) d -> p t d", p=128),
            )
            nc.gpsimd.dma_start(
                out=SYD.rearrange("p (t c) -> p t c", c=2 * D),
                in_=zyd.rearrange("(t p) c -> p t c", p=128),
            )
            nc.gpsimd.dma_start(out=Hmask[:], in_=hmd[:])
            nc.gpsimd.dma_start(out=Eall[:], in_=Ed[:])

            def SY(t):
                return SYD[:, t * 2 * D : t * 2 * D + D]

            def SXD(t):
                return SYD[:, t * 2 * D + D : (t + 1) * 2 * D]

            # dummy transpose absorbs the identity-ready wait on PE; dummy
            # matmul absorbs the staging-DMA wait
            pdum = ppp.tile([128, 128], f32, tag="tp")
            nc.tensor.transpose(pdum[:], ident[:], ident[:])
            pdm2 = ppp.tile([8, 8], f32, tag="xn")
            nc.tensor.matmul(out=pdm2[:], lhsT=Hmask[:, 0:8], rhs=Hmask[:, 0:8])
            for t in range(N // 128):
                pt = ppp.tile([128, 128], f32, tag="tp")
                nc.tensor.transpose(pt[:], SX[:, t * 128 : (t + 1) * 128], ident[:])
                nc.scalar.activation(
                    out=X2T[:, t * 128 : (t + 1) * 128], in_=pt[:], func=AF.Copy,
                    scale=-2.0,
                )
                sq = pp.tile([128, 128], f32, tag="sq")
                nc.scalar.activation(out=sq[:], in_=pt[:], func=AF.Square)
                xnp = ppp.tile([8, 128], f32, tag="xn")
                nc.tensor.matmul(out=xnp[:], lhsT=Hmask[:, 0:8], rhs=sq[:])
                nc.scalar.activation(
                    out=xna[:, t * 128 : (t + 1) * 128], in_=xnp[:],
                    func=AF.Identity, bias=half8[:], scale=1.0,
                )
            for t in range(RPC // 128):
                pt = ppp.tile([128, 128], f32, tag="tp")
                nc.tensor.transpose(pt[:], SY(t), ident[:])
                nc.scalar.activation(
                    out=YTfull[:, t * 128 : (t + 1) * 128], in_=pt[:], func=AF.Copy,
                )
                sq = pp.tile([128, 128], f32, tag="sq")
                nc.scalar.activation(out=sq[:], in_=pt[:], func=AF.Square)
                ynp = ppp.tile([8, 128], f32, tag="xn")
                nc.tensor.matmul(out=ynp[:], lhsT=Hmask[:, 0:8], rhs=sq[:])
                nc.scalar.activation(
                    out=yna[:, t * 128 : (t + 1) * 128], in_=ynp[:],
                    func=AF.Identity, bias=half8[:], scale=1.0,
                )

            # assemble packed operands
            for h in range(H):
                t, s = HT[h], HS[h]
                nc.gpsimd.dma_start(
                    out=XTA[t][32 * s : 32 * s + 16, :],
                    in_=X2T[DH * h : DH * (h + 1), :],
                )
                nc.gpsimd.dma_start(
                    out=XTA[t][32 * s + 16 : 32 * s + 17, :], in_=xna[h : h + 1, :]
                )
                nc.gpsimd.dma_start(
                    out=XTA[t][32 * s + 17 : 32 * s + 18, :], in_=onesd[:]
                )
                nc.gpsimd.dma_start(
                    out=YTA[t][32 * s : 32 * s + 16, :],
                    in_=YTfull[DH * h : DH * (h + 1), :],
                )
                nc.gpsimd.dma_start(
                    out=YTA[t][32 * s + 16 : 32 * s + 17, :], in_=onesd[:, 0:RPC]
                )
                nc.gpsimd.dma_start(
                    out=YTA[t][32 * s + 17 : 32 * s + 18, :], in_=yna[h : h + 1, :]
                )

            # diagonal path: vd_h(i) = 1 + ||zy_i - zx_i||^2 per head
            for t in range(NIB):
                dd = pp.tile([128, 128], f32, tag="dd")
                nc.vector.tensor_sub(dd[:], SY(t), SXD(t))
                nc.vector.tensor_mul(dd[:], dd[:], dd[:])
                nc.vector.tensor_reduce(
                    out=vdall[:, t * H : (t + 1) * H],
                    in_=dd.rearrange("p (h k) -> p h k", k=DH),
                    axis=AX.X, op=ALU.add,
                )
            nc.vector.tensor_scalar(
                out=vdall[:], in0=vdall[:], scalar1=1.0, scalar2=None, op0=ALU.add
            )
            nc.vector.reciprocal_approx_fast(out=wdall[:], in_=vdall[:])
            wv = wdall.rearrange("p (t c) -> p t c", c=8)
            nc.vector.tensor_mul(
                pd1.rearrange("p (t c) -> p t c", c=4), wv[:, :, 0:4], wv[:, :, 4:8]
            )
            p1v = pd1.rearrange("p (t c) -> p t c", c=4)
            nc.vector.tensor_mul(
                pd2.rearrange("p (t c) -> p t c", c=2), p1v[:, :, 0:2], p1v[:, :, 2:4]
            )
            p2v = pd2.rearrange("p (t c) -> p t c", c=2)
            nc.vector.tensor_mul(
                pdw.rearrange("p (t c) -> p t c", c=1), p2v[:, :, 0:1], p2v[:, :, 1:2]
            )

        # ---------- main loop ----------
        rp = stk.enter_context(tc.tile_pool(name="rs_psum", bufs=1, space="PSUM"))
        rsacc = rp.tile([8, 512], f32)
        with (
            tc.tile_pool(name="mm_psum", bufs=2, space="PSUM") as mp,
            tc.tile_pool(name="wpool2", bufs=3) as wp2,
            tc.tile_pool(name="upool", bufs=8) as up,
            tc.tile_pool(name="qpool", bufs=3) as qp,
        ):
            for jb in range(NJB):
                g, k = jb % 4, jb // 4
                w2t = []
                for r, heads in enumerate(RHEADS):
                    L = len(heads) * 512
                    ps = mp.tile([128, 1536], f32, tag="ps")
                    w2 = wp2.tile([128, 1536], bf16, tag="w2")
                    # tiny write absorbs the WAR wait on this w2 slot
                    nc.vector.memset(w2[0:1, 0:1], 0.0)
                    for si, h in enumerate(heads):
                        nc.tensor.matmul(
                            out=ps[:, si * 512 : (si + 1) * 512],
                            lhsT=XTA[r][32 * si : 32 * si + 18,
                                        jb * 128 : (jb + 1) * 128].bitcast(f32r),
                            rhs=YTA[r][32 * si : 32 * si + 18, :].bitcast(f32r),
                        )
                    # reciprocal straight to bf16
                    nc.vector._custom_dve(
                        RECIPROCAL_APPROX_FAST,
                        out=w2[:, 0:L], in0=ps[:, 0:L],
                        s0=RC["s0"], s1=RC["s1"], imm2=RC["imm2"],
                    )
                    # per-(i,h) row sums accumulate on the PE
                    for si, h in enumerate(heads):
                        nc.tensor.matmul(
                            out=rsacc[:],
                            lhsT=Eall[:, h * 8 : (h + 1) * 8],
                            rhs=w2[:, si * 512 : (si + 1) * 512],
                            start=(jb == 0 and h == 0),
                            stop=(jb == NJB - 1 and h == H - 1),
                            skip_group_check=True,
                        )
                    w2t.append(w2)
                # product tree over the 8 heads in bf16: L1 DVE, L2/L3 GPSIMD
                pairs = [
                    (w2t[0][:, 0:512], w2t[0][:, 512:1024]),      # h0*h1
                    (w2t[0][:, 1024:1536], w2t[1][:, 0:512]),     # h2*h3
                    (w2t[1][:, 512:1024], w2t[1][:, 1024:1536]),  # h4*h5
                    (w2t[2][:, 0:512], w2t[2][:, 512:1024]),      # h6*h7
                ]
                us = []
                for pi, (a, b) in enumerate(pairs):
                    u = up.tile([128, 512], bf16, tag="u", name=f"u{pi}")
                    if pi == 3:
                        nc.gpsimd.tensor_mul(u[:], a, b)
                    else:
                        nc.vector.tensor_mul(u[:], a, b)
                    us.append(u)
                qa = qp.tile([128, 512], bf16, tag="q")
                qb = qp.tile([128, 512], bf16, tag="q")
                nc.gpsimd.tensor_mul(qa[:], us[0][:], us[1][:])
                nc.gpsimd.tensor_mul(qb[:], us[2][:], us[3][:])
                nc.gpsimd.tensor_mul(
                    Qst[g][:, k * 512 : (k + 1) * 512], qa[:], qb[:]
                )

        # ---------- finish: rowsums, collective, sweeps, pack ----------
        with (
            tc.tile_pool(name="fin_psum", bufs=1, space="PSUM") as fp,
            tc.tile_pool(name="fin_sbuf", bufs=2) as fs,
            tc.tile_pool(name="dram", bufs=1, space="DRAM") as dp,
        ):
            nc.scalar.activation(out=rsS[:], in_=rsacc[:], func=AF.Copy)
            # diag w values, transposed to [8, RPC]
            for t in range(NIB):
                ptw = fp.tile([128, 128], f32, tag="ptw")
                nc.tensor.transpose(
                    ptw[0:8, :], wdall[:, t * 8 : (t + 1) * 8], ident[:]
                )
                nc.scalar.activation(
                    out=wdT[:, t * 128 : (t + 1) * 128], in_=ptw[0:8, :],
                    func=AF.Copy,
                )
            nc.vector.tensor_sub(rsS[:], rsS[:], wdT[:])
            # global per-head sums -> AllReduce
            nc.vector.tensor_reduce(out=Sp[:], in_=rsS[:], axis=AX.X, op=ALU.add)
            cc_in = dp.tile([8, 1], f32, tag="ccin")
            cc_out = dp.tile([8, 1], f32, tag="ccout")
            nc.sync.dma_start(out=cc_in[:], in_=Sp[:])
            nc.gpsimd.collective_compute(
                "AllReduce",
                mybir.AluOpType.add,
                replica_groups=[list(range(NCORES))],
                ins=[cc_in.opt()],
                outs=[cc_out.opt()],
            )
            nc.sync.dma_start(out=Sg[:], in_=cc_out[:])

            # blavg = mean_h ln(S_h) - ln(n(n-1)), broadcast to partitions
            nc.scalar.activation(out=Sg[:], in_=Sg[:], func=AF.Ln)
            psb1 = fp.tile([1, 1], f32, tag="psb1")
            nc.tensor.matmul(out=psb1[:], lhsT=ones8[:, 0:1], rhs=Sg[:])
            nc.scalar.activation(
                out=blavg_t[:], in_=psb1[:], func=AF.Copy, scale=1.0 / H,
                bias=-LOG_NN1,
            )
            psB = fp.tile([128, 1], f32, tag="psB")
            nc.tensor.matmul(out=psB[:], lhsT=ones1[0:1, :], rhs=blavg_t[0:1, :])
            nc.scalar.activation(out=nbl[:], in_=psB[:], func=AF.Copy, scale=-1.0)
            nc.scalar.activation(
                out=t8b[:], in_=psB[:], func=AF.Copy, scale=float(H)
            )

            # rep: sum over (i, h) of ln(rowsum)
            nc.scalar.activation(out=rsS[:], in_=rsS[:], func=AF.Ln)
            nc.vector.tensor_reduce(
                out=lnrs_red[:], in_=rsS[:], axis=AX.X, op=ALU.add
            )
            psr = fp.tile([1, 1], f32, tag="psb1")
            nc.tensor.matmul(out=psr[:], lhsT=ones8[:, 0:1], rhs=lnrs_red[:])
            nc.scalar.activation(out=repS[:], in_=psr[:], func=AF.Copy)

            # log sweep (savg = ln(prod w)) + sums
            for g in range(4):
                nc.scalar.activation(out=Qst[g][:], in_=Qst[g][:], func=AF.Ln)
                nc.vector.tensor_reduce(
                    out=slq[:, g : g + 1], in_=Qst[g][:], axis=AX.X, op=ALU.add
                )
            nc.scalar.activation(out=Ldw[:], in_=pdw[:], func=AF.Ln)

            # sigmoid + count sweeps (need blavg)
            for g in range(4):
                sj = fs.tile([128, 8 * 512], f32, tag="sj")
                nc.scalar.activation(
                    out=sj[:], in_=Qst[g][:], func=AF.Sigmoid, scale=1.0 / H,
                    bias=nbl[:], accum_out=ssig[:, g : g + 1],
                )
                cj = fs.tile([128, 8 * 512], f32, tag="cj")
                nc.vector.tensor_scalar(
                    out=cj[:], in0=Qst[g][:], scalar1=t8b[:, 0:1], scalar2=None,
                    op0=ALU.is_gt, op1=ALU.add, accum_out=scnt[:, g : g + 1],
                )
            sigd = fs.tile([128, 4], f32, tag="sigd")
            sdtmp = fs.tile([128, 1], f32, tag="sdtmp")
            nc.scalar.activation(
                out=sigd[:], in_=Ldw[:], func=AF.Sigmoid, scale=1.0 / H,
                bias=nbl[:], accum_out=sdtmp[:],
            )
            nc.vector.tensor_copy(stats[:, 4:5], sdtmp[:])
            cd4 = fs.tile([128, 4], f32, tag="cd4")
            nc.vector.tensor_scalar(
                out=cd4[:], in0=Ldw[:], scalar1=t8b[:, 0:1], scalar2=None,
                op0=ALU.is_gt, op1=ALU.add, accum_out=stats[:, 5:6],
            )

            nc.vector.tensor_reduce(
                out=stats[:, 0:1], in_=Ldw[:], axis=AX.X, op=ALU.add
            )
            nc.vector.tensor_reduce(
                out=stats[:, 1:2], in_=slq[:], axis=AX.X, op=ALU.add
            )
            nc.vector.tensor_reduce(
                out=stats[:, 2:3], in_=ssig[:], axis=AX.X, op=ALU.add
            )
            nc.vector.tensor_reduce(
                out=stats[:, 3:4], in_=scnt[:], axis=AX.X, op=ALU.add
            )

            psO = fp.tile([1, NSTAT], f32, tag="psO")
            nc.tensor.matmul(out=psO[:], lhsT=ones128[:, 0:1], rhs=stats[:])
            nc.scalar.activation(out=outrow[:], in_=psO[:], func=AF.Copy)
            nc.scalar.activation(
                out=outrow[:, 6:7], in_=repS[:, 0:1], func=AF.Copy
            )
            nc.scalar.activation(
                out=outrow[:, 7:8], in_=blavg_t[:, 0:1], func=AF.Copy
            )
            nc.sync.dma_start(out=out[:], in_=outrow[:])

    nc.compile()
    return nc


_CACHED_NC = None


def _get_nc():
    global _CACHED_NC
    if _CACHED_NC is None:
        _CACHED_NC = build_bass()
    return _CACHED_NC


def make_in_maps(z_x, z_y):
    z_x = np.ascontiguousarray(z_x, dtype=np.float32)
    z_y = np.ascontiguousarray(z_y, dtype=np.float32)
    return [
        {
            "z_x": z_x,
            "z_yd": np.ascontiguousarray(
                np.concatenate(
                    [
                        z_y[c * RPC : (c + 1) * RPC],
                        z_x[c * RPC : (c + 1) * RPC],
                    ],
                    axis=1,
                )
            ),
        }
        for c in range(NCORES)
    ]


def combine(stats, z_x, z_y):
    """stats: [NCORES, NSTAT] float; returns the 9 reference outputs."""
    st = stats.astype(np.float64)
    blavg = float(st[0, 7])
    sum_Ld = st[:, 0].sum()  # sum_i sum_h l_h(i,i)
    sum_savg_full = st[:, 1].sum()
    sig_full = st[:, 2].sum()
    cnt_full = st[:, 3].sum()
    sig_diag = st[:, 4].sum()
    cp = st[:, 5].sum()
    rep_sum = st[:, 6].sum()

    mean_pos = sum_Ld / (H * N) - blavg
    mean_neg = (sum_savg_full - sum_Ld) / (H * N * (N - 1)) - blavg
    mean_sig_pos = sig_diag / N
    mean_sig_neg = (sig_full - sig_diag) / (N * (N - 1))
    cn = cnt_full - cp
    acc = (cp + (N * (N - 1) - cn)) / (N * N)
    recall = cp / N
    tpfp = cp + cn
    precision = (cp / max(tpfp, 1.0)) if tpfp > 0 else 0.0
    rep_mean = rep_sum / (H * N) - math.log(N - 1) - blavg
    zx64 = z_x.astype(np.float64)
    zy64 = z_y.astype(np.float64)
    decay = 0.01 * (np.mean(zx64 * zx64) + np.mean(zy64 * zy64))
    loss = -mean_pos + rep_mean + decay
    return np.array(
        [
            mean_pos, mean_neg, mean_sig_pos, mean_sig_neg, acc, recall,
            precision, blavg, loss,
        ],
        dtype=np.float32,
    )


def run_on_hw(z_x, z_y, trace=False):
    from concourse.bass_utils import run_bass_kernel_spmd

    nc = _get_nc()
    res = run_bass_kernel_spmd(
        nc, make_in_maps(z_x, z_y), core_ids=list(range(NCORES)), trace=trace
    )
    stats = np.stack([r["out"][0] for r in res.results])
    return combine(stats, z_x, z_y), res


def kernel(z_x, z_y):
    out, _ = run_on_hw(z_x, z_y, trace=False)
    return out



# revision 2
# speedup vs baseline: 1.0557x; 1.0557x over previous
"""Trainium2 Bass kernel for nn_MultiHeadDensityRatioEstimator (v2).

Math restructure vs the jax reference:
  logits l_h(i,j) = -log1p(sq_h(i,j))  with sq = ||zy_i||^2+||zx_j||^2-2<zy_i,zx_j>
  exp(l_h) = 1/(1+sq_h) =: w_h   -> every logsumexp becomes a plain sum of w
  sum_h l_h = ln(prod_h w_h)     -> one log per pair instead of 8

v2 layout (8 cores, zy rows sharded 512/core, pair matrix transposed:
tiles are [128 zx-rows j, 512 zy-rows i]):
  - matmul operands (XTA/YTA packs incl. norm rows) built on HOST; the
    diagonal path (Ld, sigmoid/count diag stats) is also host-side.
  - v_h = 1+sq_h from one K=18 f32r matmul per (head, j-block)      [PE]
  - w_h = reciprocal_approx_fast(v_h) -> one [128,4096] bf16 tile    [DVE]
  - per-(i,h) row sums: ones[128,1] lhsT matmuls accumulating into
    psum rows at partition offsets {0,32,64} x 3 banks               [PE]
  - product tree prod_h w_h: strided L1/L2 on GPSIMD, L3 on DVE
  - savg = Ln(prod) stored f16 in-loop with accum_out for slq        [ACT]
  - last DEFER j-blocks' products run DVE-side under the AllReduce
  - sigmoid sweep (ACT) || count sweep (DVE f16 4x-mode) after blavg
"""

import math
import sys

import numpy as np

for _p in ("/opt/trn_rl_repo",):
    if _p not in sys.path:
        sys.path.insert(0, _p)

N = 4096
D = 128
H = 8
DH = 16
NCORES = 8
RPC = N // NCORES  # rows per core = 512
NJB = N // 128  # 32 j-blocks of 128
LOG_NN1 = float(np.log(float(N) * (N - 1)))
NSTAT = 8
# lhsT packing: head h lives in tensor HT[h], 32-partition slot HS[h]
HT = [0, 0, 0, 1, 1, 1, 2, 2]
HS = [0, 1, 2, 0, 1, 2, 0, 1]
# row-sum accumulator: head h -> psum tile RST[h], partition offset RSO[h]
RST = [0, 0, 0, 1, 1, 1, 2, 2]
RSO = [0, 32, 64, 0, 32, 64, 0, 32]
# w2 column block of head h: even heads pack the first half, odd the second,
# so every product-tree level is a contiguous elementwise multiply
WCOL = [(h % 2) * 2048 + (h // 2) * 512 for h in range(8)]
DEFER = 5  # j-blocks whose product tree runs (DVE-only) under the AllReduce


def build_bass():
    import concourse.bacc as bacc
    import concourse.tile as tile
    from concourse import mybir
    from concourse.dve_ops import RECIP_APPROX_FAST_CONSTS, RECIPROCAL_APPROX_FAST

    f32 = mybir.dt.float32
    f32r = mybir.dt.float32r
    bf16 = mybir.dt.bfloat16
    f16 = mybir.dt.float16
    AF = mybir.ActivationFunctionType
    ALU = mybir.AluOpType
    AX = mybir.AxisListType
    RC = RECIP_APPROX_FAST_CONSTS

    nc = bacc.Bacc("TRN2", num_devices=NCORES, debug=False)

    # host-packed operands
    xta = nc.dram_tensor("xta", [3 * 96, N], f32r, kind="ExternalInput")
    yta = nc.dram_tensor("yta", [3 * 96, RPC], f32r, kind="ExternalInput")
    # partition-sparse diag tiles: head h at row RSO[h], col block RST[h]
    wdsp = nc.dram_tensor("wdsp", [128, 3 * RPC], f32, kind="ExternalInput")
    wds = nc.dram_tensor("wds", [128, 3], f32, kind="ExternalInput")
    out = nc.dram_tensor("out", [1, NSTAT], f32, kind="ExternalOutput")

    from contextlib import ExitStack

    with tile.TileContext(nc) as tc, ExitStack() as stk:
        big = stk.enter_context(tc.tile_pool(name="big", bufs=1))
        small = stk.enter_context(tc.tile_pool(name="small", bufs=1))

        XTA = [big.tile([96, N], f32r, tag=f"xta{t}", name=f"XTA{t}") for t in range(3)]
        YTA = [big.tile([96, RPC], f32r, tag=f"yta{t}", name=f"YTA{t}") for t in range(3)]
        # savg storage: j-block jb lives at Qst[jb % 4][:, (jb//4)*512 : ...]
        Qst = [
            big.tile([128, 8 * RPC], f16, tag=f"qst{t}", name=f"Qst{t}")
            for t in range(4)
        ]
        WDsp = small.tile([128, 3 * RPC], f32)
        WDS = small.tile([128, 3], f32)

        ones128b = small.tile([128, 1], bf16)
        onesf = small.tile([128, 1], f32)
        ones1 = small.tile([1, 128], f32)
        slqacc = small.tile([128, NJB], f32)
        ssig = small.tile([128, 4], f32)
        scnt = small.tile([128, 4], f32)
        stats4 = small.tile([128, 4], f32)
        rsS = small.tile([128, 3 * RPC], f32)
        Ssp = small.tile([128, 3], f32)
        Sp2 = small.tile([128, 3], f32)
        Sg = small.tile([128, 3], f32)
        lnSg = small.tile([128, 3], f32)
        repsp = small.tile([128, 1], f32)
        blsum = small.tile([1, 1], f32)
        blavg_t = small.tile([1, 1], f32)
        repS = small.tile([1, 1], f32)
        nbl = small.tile([128, 1], f32)
        t8b = small.tile([128, 1], f32)
        outrow = small.tile([1, NSTAT], f32)

        nc.vector.memset(ones128b[:], 1.0)
        nc.vector.memset(onesf[:], 1.0)
        nc.vector.memset(ones1[:], 1.0)
        nc.vector.memset(rsS[:], 1.0)
        nc.vector.memset(Ssp[:], 1.0)

        # stage inputs; XTA quarters so jb 0 can start early
        nc.sync.dma_start(out=YTA[0][:], in_=yta[0:96, :])
        nc.sync.dma_start(out=YTA[1][:], in_=yta[96:192, :])
        nc.sync.dma_start(out=YTA[2][:], in_=yta[192:288, :])
        for q in range(8):
            c0, c1 = q * 512, (q + 1) * 512
            for r in range(3):
                nc.sync.dma_start(
                    out=XTA[r][:, c0:c1], in_=xta[96 * r : 96 * (r + 1), c0:c1]
                )
        nc.sync.dma_start(out=WDsp[:], in_=wdsp[:])
        nc.sync.dma_start(out=WDS[:], in_=wds[:])

        # persistent psum row-sum accumulators (3 banks, heads at rows 0/32/64)
        rp = stk.enter_context(tc.tile_pool(name="rs_psum", bufs=1, space="PSUM"))
        RS = [rp.tile([128, RPC], f32, tag=f"rs{t}", name=f"rs{t}") for t in range(3)]

        wp = stk.enter_context(tc.tile_pool(name="wpool", bufs=2 + DEFER))
        up = stk.enter_context(tc.tile_pool(name="upool", bufs=2))
        qp = stk.enter_context(tc.tile_pool(name="qpool", bufs=2))
        fs = stk.enter_context(tc.tile_pool(name="fin_sbuf", bufs=1))
        dp = stk.enter_context(tc.tile_pool(name="dram", bufs=1, space="DRAM"))

        W2 = {}

        def products(jb, eng):
            """Product tree over 8 heads for block jb; eng='gp' runs L1/L3
            on GPSIMD with L2 on DVE, 'dve' runs everything on DVE."""
            w2 = W2[jb]
            uu = up.tile([128, 2048], bf16, tag="u")
            qq = qp.tile([128, 1024], bf16, tag="q")
            QQ = qp.tile([128, 512], bf16, tag="Q")
            e1 = nc.gpsimd if eng == "gp" else nc.vector
            e1.tensor_mul(uu[:], w2[:, 0:2048], w2[:, 2048:4096])
            nc.vector.tensor_mul(qq[:], uu[:, 0:1024], uu[:, 1024:2048])
            e1.tensor_mul(QQ[:], qq[:, 0:512], qq[:, 512:1024])
            g, k = jb % 4, jb // 4
            nc.scalar.activation(
                out=Qst[g][:, k * 512 : (k + 1) * 512],
                in_=QQ[:],
                func=AF.Ln,
                accum_out=slqacc[:, jb : jb + 1],
            )

        def rowsums(jb):
            # per-(i,h) row sums accumulate on the PE; shared ones lhsT
            for h in range(H):
                nc.tensor.matmul(
                    out=RS[RST[h]][RSO[h] : RSO[h] + 1, :],
                    lhsT=ones128b[:, 0:1],
                    rhs=W2[jb][:, WCOL[h] : WCOL[h] + 512],
                    start=(jb == 0),
                    stop=(jb == NJB - 1),
                    skip_group_check=True,
                )

        with tc.tile_pool(name="mm_psum", bufs=2, space="PSUM") as mp:
            for jb in range(NJB):
                w2 = wp.tile([128, 8 * 512], bf16, tag="w2", name=f"w2_{jb}")
                W2[jb] = w2
                # tiny write absorbs the WAR wait on this w2 slot
                nc.vector.memset(w2[0:1, 0:1], 0.0)
                for g in range(4):
                    heads = (2 * g, 2 * g + 1)
                    ps = mp.tile([128, 1024], f32, tag="ps")
                    for si, h in enumerate(heads):
                        nc.tensor.matmul(
                            out=ps[:, si * 512 : (si + 1) * 512],
                            lhsT=XTA[HT[h]][
                                32 * HS[h] : 32 * HS[h] + 18,
                                jb * 128 : (jb + 1) * 128,
                            ],
                            rhs=YTA[HT[h]][32 * HS[h] : 32 * HS[h] + 18, :],
                        )
                    w2v = w2.rearrange("p (two k c) -> p two k c", two=2, c=512)
                    nc.vector._custom_dve(
                        RECIPROCAL_APPROX_FAST,
                        out=w2v[:, :, g, :],
                        in0=ps[:, 0:1024].rearrange("p (two c) -> p two c", c=512),
                        s0=RC["s0"],
                        s1=RC["s1"],
                        imm2=RC["imm2"],
                    )
                # one-block software pipeline: consume jb-1 while jb computes
                if jb > 0:
                    rowsums(jb - 1)
                    if jb - 1 < NJB - DEFER:
                        products(jb - 1, "gp")
            rowsums(NJB - 1)

        with tc.tile_pool(name="fin_psum2", bufs=1, space="PSUM") as fp2:
            # ---- S partials (CC-critical): 4 heads on DVE, 4 on ACT ----
            for h in range(H):
                t, ro, b = RST[h], RSO[h], RST[h]
                if h % 2 == 0:
                    nc.vector.tensor_reduce(
                        out=Ssp[ro : ro + 1, b : b + 1],
                        in_=RS[t][ro : ro + 1, :],
                        axis=AX.X,
                        op=ALU.add,
                    )
                else:
                    nc.scalar.activation(
                        out=rsS[ro : ro + 1, b * RPC : (b + 1) * RPC],
                        in_=RS[t][ro : ro + 1, :],
                        func=AF.Copy,
                        accum_out=Ssp[ro : ro + 1, b : b + 1],
                    )
            nc.vector.tensor_sub(Sp2[:], Ssp[:], WDS[:])
            cc_in = dp.tile([128, 3], f32, tag="ccin")
            cc_out = dp.tile([128, 3], f32, tag="ccout")
            nc.sync.dma_start(out=cc_in[:], in_=Sp2[:])
            nc.gpsimd.collective_compute(
                "AllReduce",
                mybir.AluOpType.add,
                replica_groups=[list(range(NCORES))],
                ins=[cc_in.opt()],
                outs=[cc_out.opt()],
            )

            # ---- work that hides under the AllReduce ----
            for jb in range(NJB - DEFER, NJB):
                products(jb, "dve")
            # rep: ln of off-diag row sums (partition-sparse)
            for h in range(H):
                t, ro, b = RST[h], RSO[h], RST[h]
                cs = slice(b * RPC, (b + 1) * RPC)
                if h % 2 == 0:
                    nc.vector.tensor_sub(
                        rsS[ro : ro + 1, cs], RS[t][ro : ro + 1, :],
                        WDsp[ro : ro + 1, cs],
                    )
                else:
                    nc.vector.tensor_sub(
                        rsS[ro : ro + 1, cs], rsS[ro : ro + 1, cs],
                        WDsp[ro : ro + 1, cs],
                    )
            lnrs = fs.tile([128, 3 * RPC], f32, tag="lnrs")
            nc.scalar.activation(
                out=lnrs[:], in_=rsS[:], func=AF.Ln, accum_out=repsp[:]
            )
            psr = fp2.tile([1, 1], f32, tag="psr")
            nc.tensor.matmul(out=psr[:], lhsT=onesf[:, 0:1], rhs=repsp[:])
            nc.scalar.activation(out=repS[:], in_=psr[:], func=AF.Copy)
            nc.vector.tensor_reduce(
                out=stats4[:, 0:1], in_=slqacc[:], axis=AX.X, op=ALU.add
            )

            # ---- blavg from the collective ----
            nc.sync.dma_start(out=Sg[:], in_=cc_out[:])
            nc.scalar.activation(out=lnSg[:], in_=Sg[:], func=AF.Ln)
            psb = fp2.tile([1, 3], f32, tag="psb")
            nc.tensor.matmul(out=psb[:], lhsT=onesf[:, 0:1], rhs=lnSg[:])
            nc.vector.tensor_reduce(
                out=blsum[:], in_=psb[:], axis=AX.X, op=ALU.add
            )
            nc.scalar.activation(
                out=blavg_t[:], in_=blsum[:], func=AF.Copy, scale=1.0 / H,
                bias=-LOG_NN1,
            )
            psB = fp2.tile([128, 1], f32, tag="psB")
            nc.tensor.matmul(out=psB[:], lhsT=ones1[0:1, :], rhs=blavg_t[0:1, :])
            nc.scalar.activation(out=nbl[:], in_=psB[:], func=AF.Copy, scale=-1.0)
            nc.scalar.activation(out=t8b[:], in_=psB[:], func=AF.Copy, scale=float(H))

            # ---- sigmoid (ACT) || count (DVE f16) sweeps ----
            for g in range(4):
                sj = fs.tile([128, 8 * 512], f16, tag="sj")
                nc.scalar.activation(
                    out=sj[:],
                    in_=Qst[g][:],
                    func=AF.Sigmoid,
                    scale=1.0 / H,
                    bias=nbl[:],
                    accum_out=ssig[:, g : g + 1],
                )
                cj = fs.tile([128, 8 * 512], f16, tag="cj")
                nc.vector.tensor_scalar(
                    out=cj[:],
                    in0=Qst[g][:],
                    scalar1=t8b[:, 0:1],
                    scalar2=None,
                    op0=ALU.is_gt,
                    op1=ALU.add,
                    accum_out=scnt[:, g : g + 1],
                )
            nc.vector.tensor_reduce(
                out=stats4[:, 1:2], in_=ssig[:], axis=AX.X, op=ALU.add
            )
            nc.vector.tensor_reduce(
                out=stats4[:, 2:3], in_=scnt[:], axis=AX.X, op=ALU.add
            )
            nc.vector.memset(stats4[:, 3:4], 0.0)

            psO = fp2.tile([1, 4], f32, tag="psO")
            nc.tensor.matmul(out=psO[:], lhsT=onesf[:, 0:1], rhs=stats4[:])
            nc.scalar.activation(out=outrow[:, 0:4], in_=psO[:], func=AF.Copy)
            nc.scalar.activation(out=outrow[:, 4:5], in_=repS[:, 0:1], func=AF.Copy)
            nc.scalar.activation(
                out=outrow[:, 5:6], in_=blavg_t[:, 0:1], func=AF.Copy
            )
            nc.scalar.activation(out=outrow[:, 6:8], in_=psO[:, 0:2], func=AF.Copy)
            nc.sync.dma_start(out=out[:], in_=outrow[:])

    nc.compile()
    return nc


_CACHED_NC = None


def _get_nc():
    global _CACHED_NC
    if _CACHED_NC is None:
        _CACHED_NC = build_bass()
    return _CACHED_NC


def _pack_host(z_x, z_y):
    """Host-side operand packing (numpy)."""
    zx = np.ascontiguousarray(z_x, dtype=np.float32)
    zy = np.ascontiguousarray(z_y, dtype=np.float32)
    # per-head squared norms [H, N]
    zxh = zx.reshape(N, H, DH)
    zyh = zy.reshape(N, H, DH)
    xn = np.einsum("jhd,jhd->hj", zxh, zxh)
    yn = np.einsum("ihd,ihd->hi", zyh, zyh)

    xta = np.zeros((3 * 96, N), np.float32)
    for h in range(H):
        base = 96 * HT[h] + 32 * HS[h]
        xta[base : base + 16, :] = -2.0 * zxh[:, h, :].T
        xta[base + 16, :] = xn[h] + 0.5
        xta[base + 17, :] = 1.0

    ytas = []
    wdsps = []
    wdss = []
    for c in range(NCORES):
        sl = slice(c * RPC, (c + 1) * RPC)
        ytac = np.zeros((3 * 96, RPC), np.float32)
        for h in range(H):
            base = 96 * HT[h] + 32 * HS[h]
            ytac[base : base + 16, :] = zyh[sl, h, :].T
            ytac[base + 16, :] = 1.0
            ytac[base + 17, :] = yn[h, sl] + 0.5
        ytas.append(ytac)
        # diag w values per head for this core's rows
        dd = (zyh[sl] - zxh[sl]).astype(np.float32)  # [RPC, H, DH]
        vd = 1.0 + np.einsum("ihd,ihd->hi", dd, dd).astype(np.float32)  # [H,RPC]
        wd = (1.0 / vd).astype(np.float32)
        wdsp = np.zeros((128, 3 * RPC), np.float32)
        wds = np.full((128, 3), 7.0 / 8.0, np.float32)
        for h in range(H):
            wdsp[RSO[h], RST[h] * RPC : (RST[h] + 1) * RPC] = wd[h]
            wds[RSO[h], RST[h]] = wd[h].sum(dtype=np.float64)
        wdsps.append(wdsp)
        wdss.append(wds)
    return xta, ytas, wdsps, wdss


def make_in_maps(z_x, z_y):
    xta, ytas, wdsps, wdss = _pack_host(z_x, z_y)
    return [
        {
            "xta": xta,
            "yta": np.ascontiguousarray(ytas[c]),
            "wdsp": wdsps[c],
            "wds": wdss[c],
        }
        for c in range(NCORES)
    ]


def combine(stats, z_x, z_y):
    """stats: [NCORES, NSTAT]; cols = [slq, sig, cnt, 0, rep, blavg, ...].
    Host handles the diagonal path exactly in f64."""
    st = stats.astype(np.float64)
    blavg = float(st[0, 5])
    slq = st[:, 0].sum()
    sig = st[:, 1].sum()
    cnt = st[:, 2].sum()
    rep = st[:, 4].sum()

    zx64 = np.asarray(z_x, np.float64)
    zy64 = np.asarray(z_y, np.float64)
    d = (zy64 - zx64).reshape(N, H, DH)
    vd = 1.0 + np.einsum("ihd,ihd->hi", d, d)  # [H, N]
    Ld = -np.log(vd).sum(axis=0)  # [N]
    sum_Ld = Ld.sum()
    sig_diag = (1.0 / (1.0 + np.exp(-(Ld / H - blavg)))).sum()
    cp = float((Ld > H * blavg).sum())

    mean_pos = sum_Ld / (H * N) - blavg
    mean_neg = (slq - sum_Ld) / (H * N * (N - 1)) - blavg
    mean_sig_pos = sig_diag / N
    mean_sig_neg = (sig - sig_diag) / (N * (N - 1))
    cn = cnt - cp
    acc = (cp + (N * (N - 1) - cn)) / (N * N)
    recall = cp / N
    tpfp = cp + cn
    precision = (cp / max(tpfp, 1.0)) if tpfp > 0 else 0.0
    rep_mean = rep / (H * N) - math.log(N - 1) - blavg
    decay = 0.01 * (np.mean(zx64 * zx64) + np.mean(zy64 * zy64))
    loss = -mean_pos + rep_mean + decay
    return np.array(
        [
            mean_pos, mean_neg, mean_sig_pos, mean_sig_neg, acc, recall,
            precision, blavg, loss,
        ],
        dtype=np.float32,
    )


def run_on_hw(z_x, z_y, trace=False):
    from concourse.bass_utils import run_bass_kernel_spmd

    nc = _get_nc()
    res = run_bass_kernel_spmd(
        nc, make_in_maps(z_x, z_y), core_ids=list(range(NCORES)), trace=trace
    )
    stats = np.stack([r["out"][0] for r in res.results])
    return combine(stats, z_x, z_y), res


def kernel(z_x, z_y):
    out, _ = run_on_hw(z_x, z_y, trace=False)
    return out
